# revision 1
# baseline (speedup 1.0000x reference)
"""MHA kernel for Trainium2, 8 NeuronCores.

Problem: B=4, T=2048, D=1024, H=16, HD=64 fp32 multi-head attention
  qkv = x @ w_qkv ; attention per head ; out = y @ w_o

Sharding: core c handles batch b = c//2 and head-group g = c%2 (8 of the 16
heads). Each core computes its 8 heads' attention output projected through
the matching w_o row-slice, producing a partial [T, D] output; the host sums
the two partials per batch (row-parallel output projection).

Per-core pipeline (all matmuls on PE in float32r, 1 cycle/row):
  XT = x.T                (PE transpose via identity)
  QT/KT = (w_q|w_k).T xT  ([j, t] orientation, f16)
  V = x w_v               ([t, j] natural, f16, with interleaved ones cols)
  per head, per t-block:
    scoresT[s, t] = KT.T QT    (f16 in, f32 psum)
    expT = exp(0.125 * scoresT)  (ACT, f16 out)
    yu[65, t] = [V|1].T expT     (psum accumulate over s; row 64 = denom)
    y = yu[0:64] * broadcast(1/denom)   (DVE + PE broadcast)
  out = Y.T w_o            (accumulate 4 dy-chunks in psum)
"""
import sys

if "/opt/trn_rl_repo" not in sys.path:
    sys.path.insert(0, "/opt/trn_rl_repo")

import numpy as np

import concourse.bass as bass
import concourse.mybir as mybir
import concourse.tile as tile
from concourse import bacc
from concourse.bass_utils import run_bass_kernel_spmd
from concourse.masks import make_identity

T = 2048
D = 1024
NH = 8          # heads per core
HD = 64
KC = D // 128   # 8 contraction chunks
TT = T // 128   # 16 t/s tiles
F32 = mybir.dt.float32
F32R = mybir.dt.float32r
F16 = mybir.dt.float16

_CACHE = {}


def build_nc():
    nc = bacc.Bacc(
        "TRN2",
        target_bir_lowering=False,
        debug=False,
        enable_asserts=False,
        num_devices=8,
    )
    x_d = nc.dram_tensor("x", [T, D], F32R, kind="ExternalInput")
    wq_d = nc.dram_tensor("wq", [D, 512], F32R, kind="ExternalInput")
    wk_d = nc.dram_tensor("wk", [D, 512], F32R, kind="ExternalInput")
    wv_d = nc.dram_tensor("wv", [D, 512], F32R, kind="ExternalInput")
    wo_d = nc.dram_tensor("wo", [512, D], F16, kind="ExternalInput")
    out_d = nc.dram_tensor("out", [T, D], F32, kind="ExternalOutput")

    x_ap = x_d.ap()                                        # [2048, 1024]
    wq_ap = wq_d.ap().rearrange("(kc p) j -> p kc j", p=128)   # [128, 8, 512]
    wk_ap = wk_d.ap().rearrange("(kc p) j -> p kc j", p=128)
    wv_ap = wv_d.ap().rearrange("(kc p) j -> p kc j", p=128)
    wo_ap = wo_d.ap().rearrange("(c p) n -> p c n", p=128)     # [128, 4, 1024]

    with tile.TileContext(nc) as tc:
        with tc.sbuf_pool(name="persist", bufs=1) as pers:
            # ---- persistent sbuf tensors ----
            qkt = pers.tile([128, 8, T], F16)        # QT (jt 0-3) / KT (jt 4-7)
            vones = pers.tile([128, TT, NH * 65], F16)
            yt = pers.tile([128, 4, T], F16)         # normalized Y^T

            ones_f32 = pers.tile([1, HD], F32)
            nc.vector.memset(ones_f32, 1.0)
            ones_col = pers.tile([1, HD], F32R)
            nc.vector.tensor_copy(out=ones_col, in_=ones_f32)

            # ones columns of vones (col 64 of each head's 65-col group)
            vones_h = vones.rearrange("p s (h c) -> p s h c", c=65)
            nc.vector.memset(vones_h[:, :, :, 64:65], 1.0)

            # ---- phase 0 + 1: XT, then QKT / V ----
            with (
                tc.sbuf_pool(name="sb01", bufs=1) as sb01,
                tc.psum_pool(name="ps01", bufs=1) as ps01,
            ):
                ident_f32 = sb01.tile([128, 128], F32)
                make_identity(nc, ident_f32)
                ident = sb01.tile([128, 128], F32R)
                nc.vector.tensor_copy(out=ident, in_=ident_f32)

                xt = sb01.tile([128, KC, T], F32R)   # x.T  [d, t]
                wqk_sb = sb01.tile([128, KC, 1024], F32R)
                wv_sb = sb01.tile([128, KC, 512], F32R)
                nc.sync.dma_start(out=wqk_sb[:, :, 0:512], in_=wq_ap)
                nc.sync.dma_start(out=wqk_sb[:, :, 512:1024], in_=wk_ap)
                nc.sync.dma_start(out=wv_sb, in_=wv_ap)

                for tt in range(TT):
                    x_nat = sb01.tile([128, D], F32R, tag="xnat", bufs=2)
                    nc.sync.dma_start(
                        out=x_nat, in_=x_ap[tt * 128:(tt + 1) * 128, :]
                    )
                    for q in range(2):
                        xt_ps = ps01.tile([128, 512], F32R, tag="xt", bufs=2)
                        for r in range(4):
                            kc = 4 * q + r
                            nc.tensor.transpose(
                                xt_ps[:, r * 128:(r + 1) * 128],
                                x_nat[:, kc * 128:(kc + 1) * 128],
                                ident,
                            )
                        nc.vector.tensor_copy(
                            out=xt[:, 4 * q:4 * q + 4, tt * 128:(tt + 1) * 128],
                            in_=xt_ps.rearrange("p (r t) -> p r t", t=128),
                        )

                # QT / KT : [j, t] via lhsT = w chunk, rhs = XT chunk
                for jt in range(8):
                    for tb in range(4):
                        qk_ps = ps01.tile([128, 512], F32, tag="qk", bufs=3)
                        for kc in range(KC):
                            nc.tensor.matmul(
                                qk_ps,
                                wqk_sb[:, kc, jt * 128:(jt + 1) * 128],
                                xt[:, kc, tb * 512:(tb + 1) * 512],
                                start=(kc == 0),
                                stop=(kc == KC - 1),
                            )
                        nc.vector.tensor_copy(
                            out=qkt[:, jt, tb * 512:(tb + 1) * 512], in_=qk_ps
                        )

                # V natural [t, j] via lhsT = XT chunk, rhs = wv chunk
                vones_v = vones_h[:, :, :, 0:64]   # [128, TT, 8, 64]
                for tt in range(TT):
                    v_ps = ps01.tile([128, 512], F32, tag="v", bufs=3)
                    for kc in range(KC):
                        nc.tensor.matmul(
                            v_ps,
                            xt[:, kc, tt * 128:(tt + 1) * 128],
                            wv_sb[:, kc, :],
                            start=(kc == 0),
                            stop=(kc == KC - 1),
                        )
                    nc.vector.tensor_copy(
                        out=vones_v[:, tt],
                        in_=v_ps.rearrange("p (h c) -> p h c", c=64),
                    )

            # ---- phase 2: attention per head ----
            with (
                tc.sbuf_pool(name="sb2", bufs=1) as sb2,
                tc.psum_pool(name="ps2", bufs=1) as ps2,
            ):
                for h in range(NH):
                    pb = 64 * (h % 2)
                    qt_h = qkt[pb:pb + 64, h // 2, :]
                    kt_h = qkt[pb:pb + 64, 4 + h // 2, :]
                    for tb in range(2):          # t blocks of 1024
                        t0 = tb * 1024
                        yu_ps = ps2.tile([65, 1024], F32, tag="yu", bufs=2)

                        def yu_mm(j):
                            for u in range(2):
                                nc.tensor.matmul(
                                    yu_ps[:, u * 512:(u + 1) * 512],
                                    vones_h[:, j, h, :],
                                    exp_tiles[j][:, u * 512:(u + 1) * 512],
                                    start=(j == 0),
                                    stop=(j == TT - 1),
                                    skip_group_check=True,
                                )

                        exp_tiles = []
                        for i in range(TT):
                            sc_ps = ps2.tile([128, 1024], F32, tag="sc", bufs=2)
                            for u in range(2):
                                nc.tensor.matmul(
                                    sc_ps[:, u * 512:(u + 1) * 512],
                                    kt_h[:, i * 128:(i + 1) * 128],
                                    qt_h[:, t0 + u * 512:t0 + (u + 1) * 512],
                                    start=True,
                                    stop=True,
                                )
                            e_sb = sb2.tile([128, 1024], F16, tag="exp", bufs=20)
                            nc.scalar.activation(
                                e_sb, sc_ps,
                                mybir.ActivationFunctionType.Exp,
                                scale=0.125,
                            )
                            exp_tiles.append(e_sb)
                            # interleave att@v accumulation one s-chunk behind
                            # the scores stream so PE never idles waiting on
                            # ACT (keeps HAM at full clock)
                            if i >= 1:
                                yu_mm(i - 1)
                        yu_mm(TT - 1)

                        rec = sb2.tile([1, 1024], F32R, tag="rec", bufs=2)
                        with nc.allow_low_precision(reason="f32r recip"):
                            nc.vector.reciprocal(rec, yu_ps[64:65, :])
                        bc_full = ps2.tile([65, 1024], F32, tag="yu", bufs=2)
                        bc_ps = bc_full[0:64, :]
                        for u in range(2):
                            nc.tensor.matmul(
                                bc_ps[:, u * 512:(u + 1) * 512],
                                ones_col,
                                rec[:, u * 512:(u + 1) * 512],
                                start=True,
                                stop=True,
                            )
                        rec_bc = sb2.tile([64, 1024], F32, tag="recbc", bufs=2)
                        nc.vector.tensor_copy(out=rec_bc, in_=bc_ps)
                        with nc.allow_low_precision(reason="f32r y"):
                            nc.vector.tensor_mul(
                                out=yt[pb:pb + 64, h // 2, t0:t0 + 1024],
                                in0=yu_ps[0:64, :],
                                in1=rec_bc,
                            )

            # ---- phase 3: output projection ----
            with (
                tc.sbuf_pool(name="sb3", bufs=1) as sb3,
                tc.psum_pool(name="ps3", bufs=1) as ps3,
            ):
                wo_sb = sb3.tile([128, 4, D], F16)
                nc.sync.dma_start(out=wo_sb, in_=wo_ap)
                for tt in range(TT):
                    o_ps = ps3.tile([128, D], F32, tag="ops", bufs=3)
                    for c4 in range(4):
                        for u in range(2):
                            nc.tensor.matmul(
                                o_ps[:, u * 512:(u + 1) * 512],
                                yt[:, c4, tt * 128:(tt + 1) * 128],
                                wo_sb[:, c4, u * 512:(u + 1) * 512],
                                start=(c4 == 0),
                                stop=(c4 == 3),
                            )
                    o_sb = sb3.tile([128, D], F32, tag="osb", bufs=3)
                    nc.vector.tensor_copy(out=o_sb, in_=o_ps)
                    nc.sync.dma_start(
                        out=out_d.ap()[tt * 128:(tt + 1) * 128, :], in_=o_sb
                    )

    nc.compile()
    return nc


def make_in_maps(x, w_qkv, w_o):
    in_maps = []
    for c in range(8):
        b, g = c // 2, c % 2
        in_maps.append({
            "x": np.ascontiguousarray(x[b], dtype=np.float32),
            "wq": np.ascontiguousarray(
                w_qkv[:, 512 * g:512 * (g + 1)], dtype=np.float32),
            "wk": np.ascontiguousarray(
                w_qkv[:, 1024 + 512 * g:1024 + 512 * (g + 1)], dtype=np.float32),
            "wv": np.ascontiguousarray(
                w_qkv[:, 2048 + 512 * g:2048 + 512 * (g + 1)], dtype=np.float32),
            "wo": np.ascontiguousarray(
                w_o[512 * g:512 * (g + 1), :], dtype=np.float16),
        })
    return in_maps


def kernel(x, w_qkv, w_o, _trace=False, _trace_kwargs=None):
    x = np.asarray(x)
    w_qkv = np.asarray(w_qkv)
    w_o = np.asarray(w_o)
    if "nc" not in _CACHE:
        _CACHE["nc"] = build_nc()
    nc = _CACHE["nc"]
    in_maps = make_in_maps(x, w_qkv, w_o)
    res = run_bass_kernel_spmd(
        nc, in_maps, core_ids=list(range(8)),
        trace=_trace, **(_trace_kwargs or {}),
    )
    out = np.empty((4, T, D), np.float32)
    for b in range(4):
        out[b] = res.results[2 * b]["out"] + res.results[2 * b + 1]["out"]
    if _trace:
        _CACHE["last_res"] = res
    return out



# revision 8
# speedup vs baseline: 1.0150x; 1.0150x over previous
"""MHA kernel for Trainium2, 8 NeuronCores — fused-pipeline version.

Problem: B=4, T=2048, D=1024, H=16, HD=64 fp32 multi-head attention
  qkv = x @ w_qkv ; attention per head ; out = y @ w_o

Sharding: core c handles batch b = c//2 and head-group g = c%2 (8 of the 16
heads). Each core computes its 8 heads' attention output projected through
the matching w_o row-slice, producing a partial [T, D] output; the host sums
the two partials per batch (row-parallel output projection).

Single fused instruction stream, paced by the ACT engine's exp throughput
(the per-core floor: 8 heads x T^2 exps on 128 lanes @ 1.2 GHz ~ 274us).
All other engines hide inside it:
  - scores: two heads of a pair run CONCURRENTLY on the PE via row tiling
    (K=64 each, tile_position rows 0/64) -> pair costs ~N cycles, not 2N.
  - att@V: two heads run concurrently via col tiling (M=64 each, cols 0/64)
    into one [128, t] psum tile (rows 0-63 = even head, 64-127 = odd head),
    which is already y^T-oriented for the output projection.
  - softmax denominators: DVE accumulates exp tiles (f16 ping-pong), a
    1-column PE matmul (ones) does the partition reduction, DVE reciprocal,
    gpsimd partition_broadcast replicates 1/denom across partitions, one DVE
    multiply normalizes straight into yt (y^T, f16).
  - QKV projection / x-transposes / output projection are emitted as "fill"
    chains in the PE slack between score/att@V matmuls, through a 2-buffer
    [128,512] psum tag, so the PE never idles and stays at 2.4 GHz.
"""
import sys

if "/opt/trn_rl_repo" not in sys.path:
    sys.path.insert(0, "/opt/trn_rl_repo")

from collections import deque

import numpy as np

import concourse.bass as bass
import concourse.mybir as mybir
import concourse.tile as tile
from concourse import bacc
from concourse.bass_utils import run_bass_kernel_spmd
from concourse.masks import make_identity

T = 2048
D = 1024
NH = 8          # heads per core
HD = 64
KC = D // 128   # 8 contraction chunks
TT = T // 128   # 16 t/s tiles
NP = NH // 2    # 4 head pairs
F32 = mybir.dt.float32
F16 = mybir.dt.float16

_CACHE = {}


def build_nc():
    nc = bacc.Bacc(
        "TRN2",
        target_bir_lowering=False,
        debug=False,
        enable_asserts=False,
        num_devices=8,
    )
    x_d = nc.dram_tensor("x", [T, D], F32, kind="ExternalInput")
    wq_d = nc.dram_tensor("wq", [D, 512], F16, kind="ExternalInput")
    wk_d = nc.dram_tensor("wk", [D, 512], F16, kind="ExternalInput")
    wv_d = nc.dram_tensor("wv", [D, 512], F16, kind="ExternalInput")
    wo_d = nc.dram_tensor("wo", [512, D], F16, kind="ExternalInput")
    out_d = nc.dram_tensor("out", [T, D], F32, kind="ExternalOutput")

    x_ap = x_d.ap()
    wq_ap = wq_d.ap().rearrange("(kc p) j -> p kc j", p=128)   # [128, 8, 512]
    wk_ap = wk_d.ap().rearrange("(kc p) j -> p kc j", p=128)
    wv_ap = wv_d.ap().rearrange("(kc p) j -> p kc j", p=128)
    wo_ap = wo_d.ap().rearrange("(c p) n -> p c n", p=128)     # [128, 4, 1024]

    with tile.TileContext(nc) as tc:
        with (
            tc.sbuf_pool(name="sb", bufs=1) as sb,
            tc.psum_pool(name="ps", bufs=1) as ps,
        ):
            # ---- persistent sbuf ----
            xt = sb.tile([128, KC, T], F16)          # x^T  [d, t]
            qkt = sb.tile([128, 8, T], F16)          # jt 0-3 Q^T, 4-7 K^T
            v_sb = sb.tile([128, TT, 512], F16)      # V [s-part, s-chunk, j]
            yt = sb.tile([128, NP, T], F16)          # y^T [dy, pair, t]
            wqk_sb = sb.tile([128, KC, 1024], F16)   # cols 0-511 wq, 512+ wk
            wv_sb = sb.tile([128, KC, 512], F16)
            wo_sb = sb.tile([128, 4, D], F16)
            ident = sb.tile([128, 128], F32)
            make_identity(nc, ident)
            ones_v = sb.tile([128, 1], F16)
            nc.vector.memset(ones_v, 1.0)
            warm = sb.tile([1, 32], F16)
            nc.vector.memset(warm, 0.0)
            # warm up the ACT exp table before the stream needs it
            nc.scalar.activation(
                warm, warm, mybir.ActivationFunctionType.Exp, scale=0.125
            )

            nc.sync.dma_start(out=wqk_sb[:, :, 0:512], in_=wq_ap)
            nc.sync.dma_start(out=wqk_sb[:, :, 512:1024], in_=wk_ap)
            nc.sync.dma_start(out=wv_sb, in_=wv_ap)
            nc.sync.dma_start(out=wo_sb, in_=wo_ap)

            # ---------- chain emitters (each = one aux-psum chain) ----------
            def t_chain(tt, q):
                """transpose x[tt*128:, q*512:(q+1)*512] -> xt chunks."""
                x_nat = sb.tile([128, D], F32, tag="xnat", bufs=2)
                if q == 0:
                    nc.sync.dma_start(
                        out=x_nat, in_=x_ap[tt * 128:(tt + 1) * 128, :]
                    )
                else:
                    x_nat = _xnat_last[0]
                _xnat_last[0] = x_nat
                aux = ps.tile([128, 512], F32, tag="aux", bufs=2)
                for r in range(4):
                    kc = 4 * q + r
                    nc.tensor.transpose(
                        aux[:, r * 128:(r + 1) * 128],
                        x_nat[:, kc * 128:(kc + 1) * 128],
                        ident,
                    )
                nc.vector.tensor_copy(
                    out=xt[:, 4 * q:4 * q + 4, tt * 128:(tt + 1) * 128],
                    in_=aux.rearrange("p (r t) -> p r t", t=128),
                )

            def qk_chain(jt, tbc):
                """qkt[:, jt, tbc*512:(tbc+1)*512] = (w chunk)^T @ xt."""
                aux = ps.tile([128, 512], F32, tag="aux", bufs=2)
                for kc in range(KC):
                    nc.tensor.matmul(
                        aux,
                        wqk_sb[:, kc, jt * 128:(jt + 1) * 128],
                        xt[:, kc, tbc * 512:(tbc + 1) * 512],
                        start=(kc == 0),
                        stop=(kc == KC - 1),
                    )
                nc.vector.tensor_copy(
                    out=qkt[:, jt, tbc * 512:(tbc + 1) * 512], in_=aux
                )

            def v_chain(p, i):
                """v_sb[:, i, 128p:128p+128] = x-chunk @ wv cols."""
                aux = ps.tile([128, 512], F32, tag="aux", bufs=2)
                a = aux[:, 0:128]
                for kc in range(KC):
                    nc.tensor.matmul(
                        a,
                        xt[:, kc, i * 128:(i + 1) * 128],
                        wv_sb[:, kc, 128 * p:128 * p + 128],
                        start=(kc == 0),
                        stop=(kc == KC - 1),
                    )
                nc.vector.tensor_copy(
                    out=v_sb[:, i, 128 * p:128 * p + 128], in_=a
                )

            def o_chain(tt, u):
                """out[tt-block, u-half] = yt^T chunks @ wo."""
                aux = ps.tile([128, 512], F32, tag="aux", bufs=2)
                for c4 in range(4):
                    nc.tensor.matmul(
                        aux,
                        yt[:, c4, tt * 128:(tt + 1) * 128],
                        wo_sb[:, c4, u * 512:(u + 1) * 512],
                        start=(c4 == 0),
                        stop=(c4 == 3),
                    )
                o_sb = sb.tile([128, 512], F32, tag="osb", bufs=2)
                nc.vector.tensor_copy(out=o_sb, in_=aux)
                nc.sync.dma_start(
                    out=out_d.ap()[
                        tt * 128:(tt + 1) * 128, u * 512:(u + 1) * 512
                    ],
                    in_=o_sb,
                )

            _xnat_last = [None]

            # fill queue: (pe_cost_cycles, emit_fn)
            fills = deque()

            def pop_fill():
                if fills:
                    _, fn = fills.popleft()
                    fn()
                    return True
                return False

            # ---------- attention stream state ----------
            sc_t = {
                "A": ps.tile([128, 1024], F32, name="sca", tag="sca", bufs=1),
                "B": ps.tile([128, 1024], F32, name="scb", tag="scb", bufs=1),
            }
            yu_t = [None]  # current pair's [128,1024] psum tile

            exp_t = {}   # (h, i) -> sbuf exp tile, transient
            acc_t = {}   # (h, parity) -> acc tile handle

            def emit_sc(h, p, tb, i):
                """scores for head h (A=even, B=odd) of pair p, s-chunk i."""
                pb = 0 if h == "A" else 64
                sc = sc_t[h]
                for u in range(2):
                    nc.tensor.matmul(
                        sc[:, u * 512:(u + 1) * 512],
                        qkt[pb:pb + 64, 4 + p, i * 128:(i + 1) * 128],
                        qkt[pb:pb + 64, p,
                            tb * 1024 + u * 512:tb * 1024 + (u + 1) * 512],
                        start=True,
                        stop=True,
                    )

            def emit_exp(h, i):
                e = sb.tile([128, 1024], F16, tag="exp", bufs=6)
                nc.scalar.activation(
                    e, sc_t[h], mybir.ActivationFunctionType.Exp, scale=0.125
                )
                exp_t[(h, i)] = e

            def emit_acc(h, i):
                a = sb.tile([128, 1024], F16, tag="acc" + h, bufs=2)
                if i == 0:
                    nc.vector.tensor_copy(out=a, in_=exp_t[(h, i)])
                else:
                    with nc.allow_low_precision(reason="f16 exp-sum"):
                        nc.vector.tensor_add(
                            out=a, in0=acc_t[h], in1=exp_t[(h, i)]
                        )
                acc_t[h] = a

            def emit_yu(h, p, i):
                pb = 0 if h == "A" else 64
                e = exp_t.pop((h, i))
                for u in range(2):
                    nc.tensor.matmul(
                        yu_t[0][pb:pb + 64, u * 512:(u + 1) * 512],
                        v_sb[:, i, 128 * p + pb:128 * p + pb + 64],
                        e[:, u * 512:(u + 1) * 512],
                        start=(i == 0),
                        stop=(i == TT - 1),
                        skip_group_check=True,
                    )

            def emit_norm(p, tb, yu, accA, accB):
                """denominators -> reciprocal -> broadcast -> normalize."""
                dn = ps.tile([128, 512], F32, tag="aux", bufs=2)
                for row, (acc, u) in enumerate(
                    [(accA, 0), (accA, 1), (accB, 0), (accB, 1)]
                ):
                    nc.tensor.matmul(
                        dn[32 * row:32 * row + 1, :],
                        ones_v,
                        acc[:, u * 512:(u + 1) * 512],
                        start=True,
                        stop=True,
                        tile_position=(0, 32 * row),
                    )
                rec = sb.tile([1, 2048], F16, tag="rec", bufs=2)
                for row in range(4):
                    with nc.allow_low_precision(reason="f16 recip"):
                        nc.vector.reciprocal(
                            rec[0:1, 512 * row:512 * (row + 1)],
                            dn[32 * row:32 * row + 1, :],
                        )
                bc = sb.tile([128, 2048], F16, tag="recbc", bufs=2)
                nc.gpsimd.partition_broadcast(bc, rec, channels=128)
                with nc.allow_low_precision(reason="f16 y"):
                    for row, (pb, u) in enumerate(
                        [(0, 0), (0, 1), (64, 0), (64, 1)]
                    ):
                        nc.vector.tensor_mul(
                            out=yt[pb:pb + 64, p,
                                   tb * 1024 + u * 512:
                                   tb * 1024 + (u + 1) * 512],
                            in0=yu[pb:pb + 64, u * 512:(u + 1) * 512],
                            in1=bc[pb:pb + 64, 512 * row:512 * (row + 1)],
                        )

            # ---------- startup: x^T + QK for pair 0 + first V chains ----
            for tbc in range(4):
                for tt in range(4 * tbc, 4 * tbc + 4):
                    t_chain(tt, 0)
                    t_chain(tt, 1)
                qk_chain(4, tbc)   # K^T pair 0
            for tbc in range(4):
                qk_chain(0, tbc)   # Q^T pair 0
            v_chain(0, 0)
            v_chain(0, 1)

            # ---------- fused attention stream ----------
            deferred_norm = [None]

            for tb in range(2):
                for p in range(NP):
                    yu_t[0] = ps.tile(
                        [128, 1024], F32, name="yu", tag="yu", bufs=1
                    )
                    # fill sub-queues for this unit
                    if tb == 0:
                        for jt in (p + 1, 4 + p + 1) if p < 3 else ():
                            for tbc in range(4):
                                fills.append(
                                    (4096, lambda jt=jt, tbc=tbc:
                                     qk_chain(jt, tbc))
                                )
                    if tb == 1 and p == 0:
                        for tt in range(8):
                            for u in range(2):
                                fills.append(
                                    (2048, lambda tt=tt, u=u: o_chain(tt, u))
                                )
                    budget = 0.0
                    for i in range(TT):
                        emit_sc("A", p, tb, i)
                        if i == 0 and deferred_norm[0] is not None:
                            emit_exp("A", i)
                            deferred_norm[0]()
                            deferred_norm[0] = None
                        else:
                            emit_exp("A", i)
                        if i >= 1:
                            emit_sc("B", p, tb, i - 1)
                            emit_exp("B", i - 1)
                        emit_acc("A", i)
                        if i >= 1:
                            emit_acc("B", i - 1)
                        if i >= 1:
                            emit_yu("A", p, i - 1)
                        if i >= 2:
                            emit_yu("B", p, i - 2)
                        # forced V lookahead for this pair (needed by yu)
                        if tb == 0 and i + 2 < TT:
                            v_chain(p, i + 2)
                        if tb == 0 and p < 3 and i >= TT - 3 and i < TT - 1:
                            v_chain(p + 1, i - (TT - 3))
                        # budget-paced fills
                        budget += 2000.0
                        while fills and budget >= fills[0][0]:
                            c, _ = fills[0]
                            pop_fill()
                            budget -= c
                    # tail
                    emit_sc("B", p, tb, TT - 1)
                    emit_exp("B", TT - 1)
                    emit_acc("B", TT - 1)
                    emit_yu("A", p, TT - 1)
                    emit_yu("B", p, TT - 2)
                    emit_yu("B", p, TT - 1)
                    deferred_norm[0] = (
                        lambda p=p, tb=tb, yu=yu_t[0],
                        aA=acc_t["A"], aB=acc_t["B"]:
                        emit_norm(p, tb, yu, aA, aB)
                    )
                # flush at tb end to release yu before next tb
                deferred_norm[0]()
                deferred_norm[0] = None

            # ---------- tail: output projection for tb=1 ----------
            for tt in range(8, 16):
                for u in range(2):
                    o_chain(tt, u)
            while fills:
                pop_fill()

    nc.compile()
    return nc


def make_in_maps(x, w_qkv, w_o):
    in_maps = []
    for c in range(8):
        b, g = c // 2, c % 2
        in_maps.append({
            "x": np.ascontiguousarray(x[b], dtype=np.float32),
            "wq": np.ascontiguousarray(
                w_qkv[:, 512 * g:512 * (g + 1)], dtype=np.float16),
            "wk": np.ascontiguousarray(
                w_qkv[:, 1024 + 512 * g:1024 + 512 * (g + 1)],
                dtype=np.float16),
            "wv": np.ascontiguousarray(
                w_qkv[:, 2048 + 512 * g:2048 + 512 * (g + 1)],
                dtype=np.float16),
            "wo": np.ascontiguousarray(
                w_o[512 * g:512 * (g + 1), :], dtype=np.float16),
        })
    return in_maps


def kernel(x, w_qkv, w_o, _trace=False, _trace_kwargs=None):
    x = np.asarray(x)
    w_qkv = np.asarray(w_qkv)
    w_o = np.asarray(w_o)
    if "nc" not in _CACHE:
        _CACHE["nc"] = build_nc()
    nc = _CACHE["nc"]
    in_maps = make_in_maps(x, w_qkv, w_o)
    res = run_bass_kernel_spmd(
        nc, in_maps, core_ids=list(range(8)),
        trace=_trace, **(_trace_kwargs or {}),
    )
    out = np.empty((4, T, D), np.float32)
    for b in range(4):
        out[b] = res.results[2 * b]["out"] + res.results[2 * b + 1]["out"]
    if _trace:
        _CACHE["last_res"] = res
    return out
